# revision 14
# baseline (speedup 1.0000x reference)
"""Trainium2 Bass kernel for a bidirectional-Mamba decoder layer.

Sharding: data-parallel over batch, one sequence per NeuronCore (B=8, 8 cores).
Layout: transposed throughout (features on partitions, time on free dim).

Schedule: pre(f) -> [scan_f d-tile i || pre(r) chunk i] -> scan_r -> out/LN/FFN.
The two directions' pre/scan phases are software-pipelined so the Tensor-bound
projection work of one direction hides under the Vector-bound scan of the other.
"""
import sys
sys.path.insert(0, "/opt/trn_rl_repo")

import functools
import numpy as np

import concourse.bass as bass
import concourse.mybir as mybir
import concourse.tile as tile
from concourse import bacc
from concourse.bass import ts
from concourse.bass_utils import run_bass_kernel_spmd
from concourse.masks import make_identity

# Restrict activation-table choice so the table-load pass doesn't ping-pong
# between equivalent tables. Index order must be preserved (the emitted
# act_func_set_id is the index into act_info.json), so unwanted tables are
# emptied in place rather than removed.
import concourse.hw_specs as _hw_specs
_KEEP_TABLES = {"natural_log_exp_and_others", "silu_and_others",
                "gelu_and_others", "sqrt_and_others"}
_orig_get_tables = _hw_specs.get_activation_tables
_tab_cache = {}


def _filtered_tables(arch):
    if arch not in _tab_cache:
        t = _orig_get_tables(arch)
        _tab_cache[arch] = {k: (v if k in _KEEP_TABLES else set()) for k, v in t.items()}
    return _tab_cache[arch]


_hw_specs.get_activation_tables = _filtered_tables
import concourse.bacc as _bacc_mod
_bacc_mod.get_activation_tables = _filtered_tables

FP32 = mybir.dt.float32
BF16 = mybir.dt.bfloat16
AOP = mybir.AluOpType
AF = mybir.ActivationFunctionType

DM, DI, DS, DTR, DFF, L = 512, 1024, 16, 32, 2048, 512
NDM, NDI, NFF = DM // 128, DI // 128, DFF // 128   # 4, 8, 16
NB = 8  # batch == cores
NQ = 4  # n-values per fused scan quad

N_GPS = 2  # dBu quads (out of 4) computed on GpSimd instead of DVE
# d-tiles adjacent to GpSimd broadcast bursts run dBu fully on DVE so the
# in-order GpSimd queue can drain the broadcasts without stalling the scan.
N_GPS_OVERRIDE = {("f", 5): 0, ("f", 6): 1, ("r", 0): 0, ("r", 1): 1}

W_SHAPES = {}
for p in ("f", "r"):
    W_SHAPES.update({
        p + "_in_w": (2 * DI, DM), p + "_conv_w": (DI, 4), p + "_conv_b": (DI,),
        p + "_xproj_w": (DTR + 2 * DS, DI), p + "_dt_w": (DI, DTR), p + "_dt_b": (DI,),
        p + "_A_log": (DI, DS), p + "_D": (DI,), p + "_out_w": (DM, DI),
    })
W_SHAPES.update({
    "conv1_w": (DFF, DM), "conv1_b": (DFF,), "conv2_w": (DM, DFF), "conv2_b": (DM,),
    "ln1_g": (DM,), "ln1_b": (DM,), "ln2_g": (DM,), "ln2_b": (DM,),
})
T_SHAPES = {}
for p in ("f", "r"):
    T_SHAPES.update({
        p + "_in_wT": (DM, 2 * DI), p + "_xproj_wT": (DI, DTR + 2 * DS),
        p + "_dt_wT": (DTR, DI), p + "_out_wT": (DI, DM),
    })
T_SHAPES.update({"conv1_wT": (DM, DFF), "conv2_wT": (DFF, DM)})
T_SOURCES = {n: n[:-1] for n in T_SHAPES}  # strip trailing T -> source weight name


class Ctx:
    """Per-kernel emission context (tiles shared across phases)."""
    pass


def _col_tile(nc, pool, ap_1d, n_cols, name, engine=None):
    """Load a (n_cols*128,) DRAM tensor into a [128, n_cols] SBUF tile,
    column c = elements [c*128, (c+1)*128)."""
    t = pool.tile([128, n_cols], FP32, name=name, tag="b_" + name, bufs=1)
    src = ap_1d.rearrange("(o p) -> p o", p=128)
    (engine or nc.gpsimd).dma_start(t, src)
    return t


def _pre_in_proj(cx, pfx, mi_lo, mi_hi):
    """in_proj out-tiles [mi_lo, mi_hi): u half -> u_pad (Copy), z half -> silu_z."""
    nc = cx.nc
    in_wT = cx.ins[pfx + "_in_wT"]  # [DM, 2DI] bf16 host-transposed
    src = in_wT.rearrange("(k p) m -> p k m", p=128)  # [128, 4, 2048]
    for mi in range(mi_lo, mi_hi):
        w = cx.wpool.tile([128, NDM, 128], BF16, name=f"w_in_{pfx}_{mi}", tag="wk", bufs=2)
        nc.sync.dma_start(w, src[:, :, ts(mi, 128)])
        ps = cx.pwork.tile([128, L], FP32, name=f"ps_in_{pfx}_{mi}", tag="work")
        for ki in range(NDM):
            nc.tensor.matmul(ps, w[:, ki], cx.xTb[ki],
                             start=(ki == 0), stop=(ki == NDM - 1))
        if mi < NDI:
            up = cx.bpool.tile([128, L + 6], BF16, name=f"u_pad_{pfx}_{mi}", tag="u_pad", bufs=8)
            nc.gpsimd.memset(up[:, 0:3], 0)
            nc.gpsimd.memset(up[:, L + 3:L + 6], 0)
            nc.scalar.activation(up[:, 3:L + 3], ps, AF.Copy)
            cx.u_pad[pfx].append(up)
        else:
            zi = mi - NDI
            sz = cx.bpool.tile([128, L], BF16, name=f"silu_z_{pfx}_{zi}", tag="sz", bufs=10)
            nc.scalar.activation(sz, ps, AF.Silu)
            cx.silu_z[pfx].append(sz)


def _pre_conv(cx, pfx, di_lo, di_hi, rev):
    """Causal depthwise conv (PE diag matmuls) + native-Silu -> u."""
    nc = cx.nc
    for di in range(di_lo, di_hi):
        ps = cx.pwork.tile([128, L], FP32, name=f"ps_cv_{pfx}_{di}", tag="work")
        for j in range(4):
            dg = cx.wpool.tile([128, 128], BF16, name=f"dg_{pfx}_{di}_{j}", tag="dg", bufs=6)
            jj = j if not rev else 3 - j
            nc.vector.tensor_scalar_mul(dg, cx.ident, cx.wc[pfx][:, di * 4 + jj:di * 4 + jj + 1])
            if not rev:
                rhs = cx.u_pad[pfx][di][:, jj:jj + L]
            else:
                rhs = cx.u_pad[pfx][di][:, 3 + j:3 + j + L]
            nc.tensor.matmul(ps, dg, rhs, start=(j == 0), stop=(j == 3))
        ut = cx.bpool.tile([128, L], BF16, name=f"u_{pfx}_{di}", tag="u", bufs=10)
        nc.scalar.activation(ut, ps, AF.Silu, bias=cx.conv_b[pfx][:, di:di + 1])
        cx.u[pfx].append(ut)


def _pre_xproj(cx, pfx):
    nc = cx.nc
    xproj_wT = cx.ins[pfx + "_xproj_wT"]  # [DI, 64]
    xpw = cx.wpool.tile([128, NDI * 64], BF16, name=f"xpw_{pfx}", tag="xpw", bufs=2)
    nc.sync.dma_start(xpw.rearrange("p (k m) -> p k m", m=64),
                      xproj_wT.rearrange("(k p) m -> p k m", p=128))
    ps = cx.pwork.tile([64, L], FP32, name=f"ps_dbc_{pfx}", tag="work")
    for ki in range(NDI):
        nc.tensor.matmul(ps, xpw[:, ts(ki, 64)], cx.u[pfx][ki],
                         start=(ki == 0), stop=(ki == NDI - 1))
    dbc = cx.bpool.tile([64, L], BF16, name=f"dbc_{pfx}", tag="dbc", bufs=2)
    nc.scalar.activation(dbc, ps, AF.Copy)
    cx.dbc[pfx] = dbc


def _pre_dt_du(cx, pfx):
    """dt_proj + softplus -> delta; du = delta*u (GpSimd); A = -exp(A_log)."""
    nc = cx.nc
    dtw = cx.wpool.tile([DTR, DI], BF16, name=f"dtw_{pfx}", tag="dtw", bufs=2)
    nc.sync.dma_start(dtw, cx.ins[pfx + "_dt_wT"])
    # A = -exp(A_log): one [128, 8*16] tile per direction
    al = cx.wpool.tile([128, NDI * DS], FP32, name=f"alog_{pfx}", tag="alog", bufs=2)
    nc.gpsimd.dma_start(al.rearrange("p (o n) -> p o n", n=DS),
                        cx.ins[pfx + "_A_log"].rearrange("(o p) n -> p o n", p=128))
    ae = cx.wpool.tile([128, NDI * DS], FP32, name=f"ae_{pfx}", tag="ae", bufs=2)
    nc.scalar.activation(ae, al, AF.Exp)
    A = cx.wpool.tile([128, NDI * DS], FP32, name=f"A_{pfx}", tag="A", bufs=2)
    nc.vector.tensor_scalar_mul(A, ae, -1.0)
    cx.A[pfx] = A
    for di in range(NDI):
        ps = cx.pwork.tile([128, L], FP32, name=f"ps_dt_{pfx}_{di}", tag="work")
        nc.tensor.matmul(ps, dtw[:, ts(di, 128)], cx.dbc[pfx][0:DTR, :], start=True, stop=True)
        ed = cx.spool.tile([128, L], BF16, name=f"ed_{pfx}_{di}", tag="ed", bufs=1)
        nc.scalar.activation(ed, ps, AF.Exp, bias=cx.dt_b[pfx][:, di:di + 1])
        dl = cx.bpool.tile([128, L], BF16, name=f"delta_{pfx}_{di}", tag="delta", bufs=10)
        nc.scalar.activation(dl, ed, AF.Ln, bias=cx.ones_col)
        cx.delta[pfx].append(dl)
        dut = cx.bpool.tile([128, L], BF16, name=f"du_{pfx}_{di}", tag="du", bufs=10)
        nc.gpsimd.tensor_mul(dut, dl, cx.u[pfx][di])
        cx.du[pfx].append(dut)


def _pre_bc(cx, pfx, which):
    """Broadcast B or C rows of dbc to 128 partitions, n-major slabs.

    B and C are emitted separately so the GpSimd in-order queue never holds a
    broadcast that waits on a scan-phase consumer ahead of scan-phase work.
    """
    nc = cx.nc
    dbc = cx.dbc[pfx]
    if which == "B":
        rep = cx.bpool.tile([128, DS * L], BF16, name=f"Brep_{pfx}", tag="Brep", bufs=2)
        off = DTR
        cx.Brep[pfx] = rep
    else:
        rep = cx.bpool.tile([128, DS * L], BF16, name=f"Crep_{pfx}", tag="Crep", bufs=1)
        off = DTR + DS
        cx.Crep[pfx] = rep
    for n in range(DS):
        br = cx.spool.tile([1, L], BF16, name=f"brow_{pfx}_{which}_{n}", tag="brow", bufs=2)
        nc.sync.dma_start(br, dbc[off + n:off + n + 1, :])
        nc.gpsimd.partition_broadcast(rep[:, ts(n, L)], br)


def _scan_dtile(cx, pfx, di, rev):
    """SSM scan block for one d-tile: dA, dBu, fused quad scans, hC, y-acc, gating."""
    nc = cx.nc
    delta, du, u = cx.delta[pfx][di], cx.du[pfx][di], cx.u[pfx][di]
    Brep, Crep, A = cx.Brep[pfx], cx.Crep[pfx], cx.A[pfx]
    ps_y = cx.pacc.tile([128, L], FP32, name=f"ps_y_{pfx}_{di}", tag="ffa")
    for q in range(DS // NQ):
        dAq = cx.spool.tile([128, NQ * L], BF16, name=f"dA_{pfx}_{di}_{q}", tag="dA")
        dBuq = cx.spool.tile([128, NQ * L], BF16, name=f"dBu_{pfx}_{di}_{q}", tag="dBu")
        hq = cx.spool.tile([128, NQ * L], BF16, name=f"h_{pfx}_{di}_{q}", tag="h")
        for j in range(NQ):
            n = q * NQ + j
            acol = A[:, di * DS + n:di * DS + n + 1]
            if not rev:
                # reset column at t=0 kills inter-segment carry (h0 = dBu0)
                nc.scalar.mul(dAq[:, j * L:j * L + 1], delta[:, 0:1], 0.0)
                nc.scalar.activation(dAq[:, j * L + 1:(j + 1) * L], delta[:, 1:L],
                                     AF.Exp, scale=acol)
            else:
                nc.scalar.mul(dAq[:, (j + 1) * L - 1:(j + 1) * L], delta[:, 0:1], 0.0)
                nc.scalar.activation(dAq[:, j * L:(j + 1) * L - 1], delta[:, 0:L - 1],
                                     AF.Exp, scale=acol)
        du_rep = du.unsqueeze(1).broadcast_to((128, NQ, L))
        n_gps = N_GPS_OVERRIDE.get((pfx, di), N_GPS)
        eng = nc.gpsimd if q < n_gps else nc.vector
        eng.tensor_mul(dBuq, du_rep, Brep[:, q * NQ * L:(q + 1) * NQ * L])
        if not rev:
            nc.vector.tensor_tensor_scan(hq, dAq, dBuq, 0.0, AOP.mult, AOP.add)
        else:
            nc.vector.tensor_tensor_scan(hq[:, ::-1], dAq[:, ::-1], dBuq[:, ::-1],
                                         0.0, AOP.mult, AOP.add)
        nc.vector.tensor_mul(hq, hq, Crep[:, q * NQ * L:(q + 1) * NQ * L])
        for j in range(NQ):
            n = q * NQ + j
            nc.tensor.matmul(ps_y, cx.ident, hq[:, ts(j, L)],
                             start=(n == 0), stop=(n == DS - 1))
    # y += u * D ; gate with silu(z)
    yD = cx.spool.tile([128, L], BF16, name=f"yD_{pfx}_{di}", tag="yD", bufs=1)
    nc.vector.scalar_tensor_tensor(yD, u, cx.Dcol[pfx][:, di:di + 1], ps_y, AOP.mult, AOP.add)
    ygt = cx.bpool.tile([128, L], BF16, name=f"yg_{pfx}_{di}", tag="yg", bufs=9)
    nc.vector.tensor_mul(ygt, yD, cx.silu_z[pfx][di])
    cx.yg[pfx].append(ygt)


def _out_dir(cx, pfx):
    """out_proj, residual-added in place into cx.cur tiles."""
    nc = cx.nc
    out_wT = cx.ins[pfx + "_out_wT"]  # [DI, DM]
    src = out_wT.rearrange("(k p) m -> p k m", p=128)  # [128, 8, 512]
    for mi in range(NDM):
        w = cx.wpool.tile([128, NDI, 128], BF16, name=f"w_op_{pfx}_{mi}", tag="wo", bufs=2)
        nc.sync.dma_start(w, src[:, :, ts(mi, 128)])
        ps = cx.pwork.tile([128, L], FP32, name=f"ps_op_{pfx}_{mi}", tag="work")
        for ki in range(NDI):
            nc.tensor.matmul(ps, w[:, ki], cx.yg[pfx][ki],
                             start=(ki == 0), stop=(ki == NDI - 1))
        nc.vector.scalar_tensor_tensor(cx.cur[mi], ps, 1.0, cx.cur[mi], AOP.mult, AOP.add)


def _layernorm(cx, x_tiles, g_col, b_col, name, out_tag=None):
    """LN over the partition(feature) axis of transposed tiles, via PE colsums."""
    nc = cx.nc
    ps_s = cx.pwork.tile([1, L], FP32, name=f"ps_s_{name}", tag="stat")
    ps_q = cx.pwork.tile([1, L], FP32, name=f"ps_q_{name}", tag="stat")
    for ki in range(NDM):
        xb = cx.spool.tile([128, L], BF16, name=f"xb_{name}_{ki}", tag="lnxb", bufs=2)
        nc.vector.tensor_copy(xb, x_tiles[ki])
        nc.tensor.matmul(ps_s, cx.ones_row, xb, start=(ki == 0), stop=(ki == NDM - 1))
        sq = cx.spool.tile([128, L], BF16, name=f"sq_{name}_{ki}", tag="sq", bufs=2)
        nc.scalar.activation(sq, x_tiles[ki], AF.Square)
        nc.tensor.matmul(ps_q, cx.ones_row, sq, start=(ki == 0), stop=(ki == NDM - 1))
    mean = cx.spool.tile([1, L], FP32, name=f"mean_{name}", tag="st1", bufs=1)
    nc.vector.tensor_scalar_mul(mean, ps_s, 1.0 / DM)
    msq = cx.spool.tile([1, L], FP32, name=f"msq_{name}", tag="st2", bufs=1)
    nc.vector.tensor_mul(msq, mean, mean)
    var = cx.spool.tile([1, L], FP32, name=f"var_{name}", tag="st3", bufs=1)
    nc.vector.scalar_tensor_tensor(var, ps_q, 1.0 / DM, msq, AOP.mult, AOP.subtract)
    sd = cx.spool.tile([1, L], FP32, name=f"sd_{name}", tag="st6", bufs=1)
    nc.scalar.activation(sd, var, AF.Sqrt, bias=cx.eps_col)
    istd = cx.spool.tile([1, L], FP32, name=f"istd_{name}", tag="st4", bufs=1)
    nc.vector.reciprocal(istd, sd)
    shift = cx.spool.tile([1, L], FP32, name=f"shift_{name}", tag="st5", bufs=1)
    nc.vector.tensor_mul(shift, mean, istd)
    nc.vector.tensor_scalar_mul(shift, shift, -1.0)
    istd_r = cx.spool.tile([128, L], FP32, name=f"istd_r_{name}", tag="istd_r", bufs=1)
    shift_r = cx.spool.tile([128, L], FP32, name=f"shift_r_{name}", tag="shift_r", bufs=1)
    nc.gpsimd.partition_broadcast(istd_r, istd)
    nc.gpsimd.partition_broadcast(shift_r, shift)
    out_tiles = []
    for ki in range(NDM):
        t1 = cx.spool.tile([128, L], FP32, name=f"t1_{name}_{ki}", tag="lnt1", bufs=2)
        nc.vector.tensor_mul(t1, x_tiles[ki], istd_r)
        nc.vector.tensor_add(t1, t1, shift_r)
        t3 = cx.bpool.tile([128, L], FP32, name=f"t3_{name}_{ki}",
                           tag=(out_tag or f"ln_{name}_{ki}"),
                           bufs=(2 if out_tag else 1))
        nc.scalar.activation(t3, t1, AF.Identity,
                             scale=g_col[:, ki:ki + 1], bias=b_col[:, ki:ki + 1])
        out_tiles.append(t3)
    return out_tiles


def _kernel(tc, out_d, ins):
    nc = tc.nc
    with (tc.tile_pool(name="const", bufs=1) as cpool,
          tc.tile_pool(name="big", bufs=1) as bpool,
          tc.tile_pool(name="wts", bufs=2) as wpool,
          tc.tile_pool(name="scan", bufs=2) as spool,
          tc.tile_pool(name="pwork", bufs=2, space="PSUM") as pwork,
          tc.tile_pool(name="pacc", bufs=4, space="PSUM") as pacc):

        cx = Ctx()
        cx.nc, cx.ins = nc, ins
        cx.wpool, cx.bpool, cx.spool, cx.pwork, cx.pacc = wpool, bpool, spool, pwork, pacc
        cx.u_pad = {"f": [], "r": []}
        cx.silu_z = {"f": [], "r": []}
        cx.u = {"f": [], "r": []}
        cx.delta = {"f": [], "r": []}
        cx.du = {"f": [], "r": []}
        cx.yg = {"f": [], "r": []}
        cx.dbc, cx.A, cx.Brep, cx.Crep = {}, {}, {}, {}
        cx.wc, cx.conv_b, cx.dt_b, cx.Dcol = {}, {}, {}, {}

        cx.ident = cpool.tile([128, 128], BF16, name="ident", tag="ident")
        make_identity(nc, cx.ident)
        cx.ones_col = cpool.tile([128, 1], FP32, name="ones_col", tag="ones_col")
        nc.vector.memset(cx.ones_col, 1.0)
        cx.ones_row = cpool.tile([128, 1], BF16, name="ones_row", tag="ones_row")
        nc.vector.memset(cx.ones_row, 1.0)
        cx.eps_col = cpool.tile([1, 1], FP32, name="eps", tag="eps")
        nc.vector.memset(cx.eps_col, 1e-5)

        # per-direction small constants (batched single DMAs)
        for p in ("f", "r"):
            wc = wpool.tile([128, NDI * 4], FP32, name=f"wc_{p}", tag="b_wc_" + p, bufs=1)
            nc.gpsimd.dma_start(wc.rearrange("p (o k) -> p o k", k=4),
                                ins[p + "_conv_w"].rearrange("(o p) k -> p o k", p=128))
            cx.wc[p] = wc
            cx.conv_b[p] = _col_tile(nc, wpool, ins[p + "_conv_b"], NDI, f"cb_{p}")
            cx.dt_b[p] = _col_tile(nc, wpool, ins[p + "_dt_b"], NDI, f"db_{p}")
            cx.Dcol[p] = _col_tile(nc, wpool, ins[p + "_D"], NDI, f"D_{p}")
        c1b = _col_tile(nc, wpool, ins["conv1_b"], NFF, "c1b")
        c2b = _col_tile(nc, wpool, ins["conv2_b"], NDM, "c2b")
        ln1g = _col_tile(nc, wpool, ins["ln1_g"], NDM, "ln1g")
        ln1b = _col_tile(nc, wpool, ins["ln1_b"], NDM, "ln1b")
        ln2g = _col_tile(nc, wpool, ins["ln2_g"], NDM, "ln2g")
        ln2b = _col_tile(nc, wpool, ins["ln2_b"], NDM, "ln2b")

        # x (residual base, fp32) and bf16 copy for matmuls
        cx.cur = []
        for i in range(NDM):
            xt = bpool.tile([128, L], FP32, name=f"xT_{i}", tag=f"xT{i}")
            nc.scalar.dma_start(xt, ins["xbT"][ts(i, 128), :])
            cx.cur.append(xt)
        cx.xTb = []
        for i in range(NDM):
            xtb = bpool.tile([128, L], BF16, name=f"xTb_{i}", tag=f"xTb{i}")
            nc.scalar.dma_start(xtb, ins["xbT16"][ts(i, 128), :])
            cx.xTb.append(xtb)

        # ---- pre-phase, forward direction (full)
        _pre_in_proj(cx, "f", 0, 2 * NDI)
        _pre_conv(cx, "f", 0, NDI, rev=False)
        _pre_xproj(cx, "f")
        _pre_dt_du(cx, "f")
        _pre_bc(cx, "f", "B")
        _pre_bc(cx, "f", "C")

        # ---- scan(f) interleaved with pre-phase chunks of reverse direction
        r_chunks = [
            lambda: _pre_in_proj(cx, "r", 0, NDI),
            lambda: _pre_in_proj(cx, "r", NDI, 2 * NDI),
            lambda: _pre_conv(cx, "r", 0, NDI // 2, rev=True),
            lambda: _pre_conv(cx, "r", NDI // 2, NDI, rev=True),
            lambda: _pre_xproj(cx, "r"),
            lambda: (_pre_dt_du(cx, "r"), _pre_bc(cx, "r", "B")),
        ]
        for i in range(NDI):
            if i < len(r_chunks):
                r_chunks[i]()
            _scan_dtile(cx, "f", i, rev=False)
        # C_r broadcasts reuse C_f's slot: emitted only after scan_f is done
        # with it, keeping the GpSimd queue free of forward-blocking waits.
        _pre_bc(cx, "r", "C")
        _out_dir(cx, "f")

        for i in range(NDI):
            _scan_dtile(cx, "r", i, rev=True)
        _out_dir(cx, "r")

        x1 = _layernorm(cx, cx.cur, ln1g, ln1b, "ln1")

        # ---- FFN: x1 += conv2_w @ gelu(conv1_w @ x1 + b1) + b2 (in place)
        x1b = []
        for ki in range(NDM):
            xc = bpool.tile([128, L], BF16, name=f"x1b_{ki}", tag=f"xTb{ki}")
            nc.vector.tensor_copy(xc, x1[ki])
            x1b.append(xc)
        ps2 = [pacc.tile([128, L], FP32, name=f"ps_ffn_{mi}", tag="ffa") for mi in range(NDM)]
        f1src = ins["conv1_wT"].rearrange("(k p) m -> p k m", p=128)  # [128, 4, 2048]
        for ffi in range(NFF):
            w1 = wpool.tile([128, NDM, 128], BF16, name=f"w_f1_{ffi}", tag="wf1", bufs=3)
            nc.sync.dma_start(w1, f1src[:, :, ts(ffi, 128)])
            ps1 = pwork.tile([128, L], FP32, name=f"ps_ff1_{ffi}", tag="work")
            for ki in range(NDM):
                nc.tensor.matmul(ps1, w1[:, ki], x1b[ki],
                                 start=(ki == 0), stop=(ki == NDM - 1))
            y1 = spool.tile([128, L], BF16, name=f"y1_{ffi}", tag="y1", bufs=2)
            nc.scalar.activation(y1, ps1, AF.Gelu, bias=c1b[:, ffi:ffi + 1])
            w2 = wpool.tile([128, NDM * 128], BF16, name=f"w_f2_{ffi}", tag="wf2", bufs=2)
            nc.sync.dma_start(w2, ins["conv2_wT"][ts(ffi, 128), :])
            for mi in range(NDM):
                nc.tensor.matmul(ps2[mi], w2[:, ts(mi, 128)], y1,
                                 start=(ffi == 0), stop=(ffi == NFF - 1))
        for mi in range(NDM):
            nc.vector.scalar_tensor_tensor(x1[mi], ps2[mi], 1.0, x1[mi], AOP.mult, AOP.add)
            nc.scalar.activation(x1[mi], x1[mi], AF.Identity, bias=c2b[:, mi:mi + 1])

        out_t = _layernorm(cx, x1, ln2g, ln2b, "ln2", out_tag="ln_out")
        for mi in range(NDM):
            nc.scalar.dma_start(out_d[ts(mi, 128), :], out_t[mi])


@functools.lru_cache(maxsize=1)
def _build():
    nc = bacc.Bacc("TRN2", debug=False)
    ins = {"xbT": nc.dram_tensor("xbT", (DM, L), FP32, kind="ExternalInput").ap()}
    for name, shape in W_SHAPES.items():
        ins[name] = nc.dram_tensor(name, shape, FP32, kind="ExternalInput").ap()
    for name, shape in T_SHAPES.items():
        ins[name] = nc.dram_tensor(name, shape, BF16, kind="ExternalInput").ap()
    ins["xbT16"] = nc.dram_tensor("xbT16", (DM, L), BF16, kind="ExternalInput").ap()
    out_d = nc.dram_tensor("out", (DM, L), FP32, kind="ExternalOutput").ap()
    with tile.TileContext(nc) as tc:
        _kernel(tc, out_d, ins)
    nc.compile()
    return nc


def make_in_maps(inputs):
    import ml_dtypes
    bf = ml_dtypes.bfloat16
    shared = {}
    for name in W_SHAPES:
        shared[name] = np.ascontiguousarray(inputs[name], dtype=np.float32)
    for tname, sname in T_SOURCES.items():
        shared[tname] = np.ascontiguousarray(
            np.asarray(inputs[sname], dtype=np.float32).T).astype(bf)
    in_maps = []
    for c in range(NB):
        m = dict(shared)
        xt = np.ascontiguousarray(np.asarray(inputs["x"][c], dtype=np.float32).T)
        m["xbT"] = xt
        m["xbT16"] = xt.astype(bf)
        in_maps.append(m)
    return in_maps


def kernel(**inputs):
    nc = _build()
    res = run_bass_kernel_spmd(nc, make_in_maps(inputs), list(range(NB)))
    return np.stack([res.results[c]["out"].T for c in range(NB)]).astype(np.float32)
